# revision 7
# baseline (speedup 1.0000x reference)
"""DND retrieval (episodic memory read) kernel for 8 Trainium2 NeuronCores.

Strategy: data-parallel over batch B=64 -> 8 envs per core. Each core runs the
full pipeline for its 8 envs:
  - q-side MLP chain with transposed activations [feat, b]; the wide Wq layer
    runs in bf16 with the small activations stationary
  - keys transposed on-chip via PE-transpose (fp32), cast to bf16 in the
    mandatory PSUM->SBUF copy
  - scores matmul + value matmul in bf16 (fp32 PSUM accumulation)
  - rpe modulation folded into post-matmul scaling (rpe factors out of the
    k-contraction), validity mask built on-chip from iota + step
  - softmax batched on a [64 (b*h), 1024 (l)] fp32 tile
  - value_aggregator in bf16 (T stationary), read_memory chains in fp32
MLP weights are replicated per core and streamed from HBM.
"""
from contextlib import ExitStack

import numpy as np

import concourse.bass as bass
import concourse.tile as tile
from concourse import bacc, mybir
from concourse.bass_utils import run_bass_kernel_spmd
from concourse.masks import make_identity

F32 = mybir.dt.float32
BF16 = mybir.dt.bfloat16
AF = mybir.ActivationFunctionType
OP = mybir.AluOpType

L = 1024      # episode length (memory slots)
B = 64        # total batch
BL = 8        # batch per core
KD = 512      # key size
VD = 512      # value size
H = 8         # heads
MEMB = 256    # memory state embedding
SDIM = 512    # state dim
HID = 512
RIMQ = 512
LAT = KD - MEMB
NCORES = 8
LC = L // 128         # 8 l-chunks
KC = KD // 128        # 4 k-chunks
RSQK = 1.0 / np.sqrt(np.float32(KD))

_CACHE: dict = {}


def _emit(nc: bass.Bass, tc: tile.TileContext, ctx: ExitStack, io: dict):
    pool = ctx.enter_context(tc.tile_pool(name="main", bufs=1))
    kpool = ctx.enter_context(tc.tile_pool(name="keys", bufs=4))
    vpool = ctx.enter_context(tc.tile_pool(name="vals", bufs=4))
    vbpool = ctx.enter_context(tc.tile_pool(name="valsb", bufs=12))
    ktpool = ctx.enter_context(tc.tile_pool(name="keysT", bufs=2))
    wpool = ctx.enter_context(tc.tile_pool(name="wstream", bufs=4))
    wbpool = ctx.enter_context(tc.tile_pool(name="wcast", bufs=4))
    psum = ctx.enter_context(tc.tile_pool(name="ps", bufs=2, space="PSUM"))

    ident = pool.tile([128, 128], F32)
    make_identity(nc, ident[:])

    # ---------------- Phase A: q-side MLP ---------------------------------
    state_nat = pool.tile([BL, SDIM], F32)
    nc.sync.dma_start(state_nat[:], io["state"][:])
    lat_nat = pool.tile([BL, LAT], F32)
    nc.sync.dma_start(lat_nat[:], io["lat"][:])

    def bias_tile(name, nch):
        t = pool.tile([128, nch], F32, tag="b" + name)
        nc.sync.dma_start(t[:], io[name][:])
        return t

    bst = bias_tile("b_state", 2)
    bcq1 = bias_tile("bcq1", 4)
    bcq2 = bias_tile("bcq2", 4)
    bq = bias_tile("bq", 32)

    # transpose state/lat chunks -> xT chunks [128, 8]
    def transp_small(src_ap, n_par, n_free_chunks, tag):
        outs = []
        for c in range(n_free_chunks):
            tp = psum.tile([128, BL], F32, tag="sm")
            nc.tensor.transpose(tp[:], src_ap[:, c * 128:(c + 1) * 128],
                                ident[0:n_par, 0:n_par])
            t = pool.tile([128, BL], F32, tag=tag + str(c))
            nc.vector.tensor_copy(t[:], tp[:])
            outs.append(t)
        return outs

    stateT = transp_small(state_nat, BL, SDIM // 128, "stT")   # 4 tiles
    latT = transp_small(lat_nat, BL, LAT // 128, "laT")        # 2 tiles

    # se = state @ W_state + b_state  (transposed out: [MEMB, b])
    Ws = []
    for k in range(4):
        w = wpool.tile([128, MEMB], F32, tag="Ws")
        nc.sync.dma_start(w[:], io["W_state"][k * 128:(k + 1) * 128, :])
        Ws.append(w)
    xT = []
    for j in range(MEMB // 128):
        ps = psum.tile([128, BL], F32, tag="sm")
        for k in range(4):
            nc.tensor.matmul(ps[:], Ws[k][:, j * 128:(j + 1) * 128],
                             stateT[k][:], start=(k == 0), stop=(k == 3))
        t = pool.tile([128, BL], F32, tag=f"xT{j}")
        nc.vector.tensor_scalar(out=t[:], in0=ps[:], scalar1=bst[:, j:j + 1],
                                scalar2=None, op0=OP.add)
        xT.append(t)
    xT.extend(latT)   # x = concat(se, latent): 4 chunks of 128

    # fp32 MLP layer: weights stationary, activations moving [128, 8]
    def mlp_layer(xin, w_name, b_tile, n_out_chunks, tag, wtag):
        wts = []
        for k in range(len(xin)):
            w = wpool.tile([128, n_out_chunks * 128], F32, tag=wtag)
            nc.sync.dma_start(w[:], io[w_name][k * 128:(k + 1) * 128, :])
            wts.append(w)
        outs = []
        for j in range(n_out_chunks):
            ps = psum.tile([128, BL], F32, tag="sm")
            for k in range(len(xin)):
                nc.tensor.matmul(ps[:], wts[k][:, j * 128:(j + 1) * 128],
                                 xin[k][:], start=(k == 0),
                                 stop=(k == len(xin) - 1))
            t = pool.tile([128, BL], F32, tag=f"{tag}{j}")
            nc.vector.tensor_scalar(out=t[:], in0=ps[:],
                                    scalar1=b_tile[:, j:j + 1],
                                    scalar2=None, op0=OP.add)
            outs.append(t)
        return outs

    h1T = mlp_layer(xT, "Wcq1", bcq1, HID // 128, "h1", "Wcq")
    qcT = mlp_layer(h1T, "Wcq2", bcq2, KD // 128, "qc", "Wcq")

    # q = qc @ Wq + bq, computed NATURAL [b, out] with qc stationary (bf16):
    # out[8, 512-block] accumulated over k-chunks; Wq is the moving operand.
    qcTb = []
    for k in range(KC):
        t = pool.tile([128, BL], BF16, tag=f"qcb{k}")
        nc.vector.tensor_copy(t[:], qcT[k][:])
        qcTb.append(t)
    qnat = pool.tile([BL, H * KD], F32)
    for jg in range(4):
        wts = []
        for k in range(KC):
            w = wpool.tile([128, 1024], F32, tag="Wq")
            nc.sync.dma_start(w[:], io["Wq"][k * 128:(k + 1) * 128,
                                             jg * 1024:(jg + 1) * 1024])
            wb = wbpool.tile([128, 1024], BF16, tag="Wqb")
            nc.gpsimd.tensor_copy(wb[:], w[:])
            wts.append(wb)
        for hf in range(2):
            ng = jg * 2 + hf
            ps = psum.tile([BL, 512], F32, tag="sp")
            for k in range(KC):
                nc.tensor.matmul(ps[:], qcTb[k][:],
                                 wts[k][:, hf * 512:(hf + 1) * 512],
                                 start=(k == 0), stop=(k == KC - 1))
            nc.scalar.copy(qnat[:, ng * 512:(ng + 1) * 512], ps[:])

    # qT blocks scattered into Qpad [128, 2048] bf16 with zero padding:
    # for (b, kc) the scores lhsT is Qpad[:, kc*512 + b*64 : +64] = the
    # (b', h) columns, nonzero only at b'==b.  This lets all 8 envs'
    # scores matmuls accumulate into one [64, 512] PSUM bank (out rows
    # b*8+h) without cross-partition copies.  Bias bq is added per q-col
    # (partition dim after transpose).
    Qpad = pool.tile([128, KC * BL * B], BF16)
    nc.gpsimd.memset(Qpad[:], 0.0)
    for j in range(32):
        h = j // KC
        kc = j % KC
        tp = psum.tile([128, BL], F32, tag="sm")
        nc.tensor.transpose(tp[:], qnat[:, j * 128:(j + 1) * 128],
                            ident[0:BL, 0:BL])
        base = kc * 512 + h
        nc.vector.tensor_scalar(out=Qpad[:, base:base + (BL - 1) * 72 + 1:72],
                                in0=tp[:], scalar1=bq[:, j:j + 1],
                                scalar2=None, op0=OP.add)

    # ---------------- Phase B: keys transpose + scores ---------------------
    S = pool.tile([B, L], F32)
    sp_half0 = psum.tile([B, 512], F32, tag="sp")
    sp_half1 = psum.tile([B, 512], F32, tag="sp")
    sp_halves = [sp_half0, sp_half1]
    for b in range(BL):
        KT = ktpool.tile([128, KC, L], BF16, tag="KT")
        for lc in range(LC):
            kn = kpool.tile([128, KD], F32, tag="knat")
            nc.sync.dma_start(kn[:], io["keys"][lc * 128:(lc + 1) * 128, b, :])
            tp = psum.tile([128, KC, 128], F32, tag="tp")
            for kc in range(KC):
                nc.tensor.transpose(tp[:, kc, :],
                                    kn[:, kc * 128:(kc + 1) * 128], ident[:])
            if (b * LC + lc) % 2 == 0:
                nc.vector.tensor_copy(KT[:, :, lc * 128:(lc + 1) * 128], tp[:])
            else:
                nc.scalar.copy(KT[:, :, lc * 128:(lc + 1) * 128], tp[:])
        for lh in range(2):
            for kc in range(KC):
                # lhsT is zero-padded to all 64 (b', h) columns, so every
                # matmul writes the full [64, 512] bank; one accumulation
                # group spans all (b, kc).
                nc.tensor.matmul(sp_halves[lh][:],
                                 Qpad[:, kc * 512 + b * 64:
                                      kc * 512 + (b + 1) * 64],
                                 KT[:, kc, lh * 512:(lh + 1) * 512],
                                 start=(b == 0 and kc == 0),
                                 stop=(b == BL - 1 and kc == KC - 1),
                                 skip_group_check=True)
    for lh in range(2):
        nc.vector.tensor_copy(S[:, lh * 512:(lh + 1) * 512], sp_halves[lh][:])

    # ---------------- Phase C: mask + softmax ------------------------------
    iot = pool.tile([B, L], F32)
    nc.gpsimd.iota(iot[:], pattern=[[1, L]], base=0, channel_multiplier=0,
                   allow_small_or_imprecise_dtypes=True)
    stept = pool.tile([B, 1], F32)
    nc.sync.dma_start(stept[:], io["step_rep"][:])
    valid = pool.tile([B, L], F32)
    nc.vector.tensor_scalar(out=valid[:], in0=iot[:], scalar1=stept[:, 0:1],
                            scalar2=None, op0=OP.is_lt)
    A = pool.tile([B, L], F32)
    nc.scalar.activation(A[:], valid[:], AF.Copy, bias=-1e30, scale=1e30)

    rpeT = pool.tile([BL, L], F32)
    for lc in range(LC):
        rp = pool.tile([128, BL], F32, tag="rp")
        nc.sync.dma_start(rp[:], io["rpe"][lc * 128:(lc + 1) * 128, :])
        tp = psum.tile([BL, 128], F32, tag="sm")
        nc.tensor.transpose(tp[:], rp[:], ident[:])
        nc.vector.tensor_copy(rpeT[:, lc * 128:(lc + 1) * 128], tp[:])
    selt = pool.tile([BL, B], F32)
    nc.sync.dma_start(selt[:], io["sel"][:])
    G = pool.tile([B, L], F32)
    for lh in range(2):
        gp = psum.tile([B, 512], F32, tag="sp")
        nc.tensor.matmul(gp[:], selt[:], rpeT[:, lh * 512:(lh + 1) * 512],
                         start=True, stop=True)
        nc.vector.tensor_tensor(out=G[:, lh * 512:(lh + 1) * 512], in0=gp[:],
                                in1=valid[:, lh * 512:(lh + 1) * 512],
                                op=OP.mult)

    nc.vector.tensor_tensor(out=S[:], in0=S[:], in1=G[:], op=OP.mult)
    nc.vector.tensor_tensor(out=S[:], in0=S[:], in1=A[:], op=OP.add)
    negM = pool.tile([B, 1], F32)
    nc.vector.tensor_reduce(out=negM[:], in_=S[:], op=OP.max,
                            axis=mybir.AxisListType.X, negate=True)
    E = pool.tile([B, L], F32)
    Z = pool.tile([B, 1], F32)
    nc.scalar.activation(E[:], S[:], AF.Exp, bias=negM[:, 0:1], scale=1.0,
                         accum_out=Z[:, 0:1])
    R = pool.tile([B, 1], F32)
    nc.vector.reciprocal(R[:], Z[:])
    P = pool.tile([B, L], F32)
    nc.vector.tensor_scalar(out=P[:], in0=E[:], scalar1=R[:, 0:1],
                            scalar2=None, op0=OP.mult)

    # ---------------- Phase D: prob transpose + value matmul ---------------
    PTs = []
    for lc in range(LC):
        PT = pool.tile([128, B], BF16, tag=f"PT{lc}")
        tpp = psum.tile([128, B], F32, tag="tp")
        nc.tensor.transpose(tpp[:], P[:, lc * 128:(lc + 1) * 128],
                            ident[0:B, 0:B])
        nc.vector.tensor_copy(PT[:], tpp[:])
        PTs.append(PT)

    T = pool.tile([128, VD // 128, H, BL], BF16)
    for b in range(BL):
        rps = psum.tile([BL, VD], F32, tag="sp")
        for lc in range(LC):
            vn = vpool.tile([128, VD], F32, tag="vnat")
            nc.sync.dma_start(vn[:], io["vals"][lc * 128:(lc + 1) * 128, b, :])
            vb = vbpool.tile([128, VD], BF16, tag="vb")
            nc.gpsimd.tensor_copy(vb[:], vn[:])
            nc.tensor.matmul(rps[:], PTs[lc][:, b * H:(b + 1) * H], vb[:],
                             start=(lc == 0), stop=(lc == LC - 1),
                             skip_group_check=True)
        rs = pool.tile([BL, VD], F32, tag="rs")
        nc.scalar.copy(rs[:], rps[:])
        for vs in range(VD // 128):
            tr = psum.tile([128, BL], F32, tag="sm")
            nc.tensor.transpose(tr[:], rs[:, vs * 128:(vs + 1) * 128],
                                ident[0:BL, 0:BL])
            nc.vector.tensor_copy(T[:, vs, :, b], tr[:])

    # ---------------- Phase E: output MLP chain ----------------------------
    bagg = bias_tile("bagg", 4)
    brk1 = bias_tile("brk1", 4)
    brk2 = bias_tile("brk2", 4)
    brv1 = bias_tile("brv1", 4)
    brv2 = bias_tile("brv2", 4)

    n_hv = (H * VD) // 128  # 32
    aggp = psum.tile([BL, VD], F32, tag="sp")
    for c in range(n_hv):
        wg = wpool.tile([128, VD], F32, tag="Wagg")
        nc.sync.dma_start(wg[:], io["Wagg"][c * 128:(c + 1) * 128, :])
        wgb = wbpool.tile([128, VD], BF16, tag="Waggb")
        nc.gpsimd.tensor_copy(wgb[:], wg[:])
        h = c // (VD // 128)
        vs = c % (VD // 128)
        nc.tensor.matmul(aggp[:], T[:, vs, h, :], wgb[:],
                         start=(c == 0), stop=(c == n_hv - 1))
    agg_nat = pool.tile([BL, VD], F32)
    nc.scalar.copy(agg_nat[:], aggp[:])
    AT = []
    for j in range(4):
        tp = psum.tile([128, BL], F32, tag="sm")
        nc.tensor.transpose(tp[:], agg_nat[:, j * 128:(j + 1) * 128],
                            ident[0:BL, 0:BL])
        t = pool.tile([128, BL], F32, tag=f"AT{j}")
        nc.vector.tensor_scalar(out=t[:], in0=tp[:], scalar1=bagg[:, j:j + 1],
                                scalar2=None, op0=OP.add)
        AT.append(t)

    hkT = mlp_layer(AT, "Wrk1", brk1, HID // 128, "hk", "Wchain")
    okT = mlp_layer(hkT, "Wrk2", brk2, RIMQ // 128, "ok", "Wchain")
    hvT = mlp_layer(AT, "Wrv1", brv1, HID // 128, "hv", "Wchain")
    ovT = mlp_layer(hvT, "Wrv2", brv2, VD // 128, "ov", "Wchain")

    for name, tiles_, width in (("out_key", okT, RIMQ), ("out_val", ovT, VD)):
        onat = pool.tile([BL, width], F32, tag="o" + name)
        for j in range(width // 128):
            tp = psum.tile([BL, 128], F32, tag="sm")
            nc.tensor.transpose(tp[:], tiles_[j][:], ident[:])
            nc.scalar.copy(onat[:, j * 128:(j + 1) * 128], tp[:])
        nc.sync.dma_start(io[name][:], onat[:])


def _build():
    nc = bacc.Bacc("TRN2", target_bir_lowering=False, debug=False,
                   num_devices=NCORES)
    io = {}

    def din(name, shape):
        io[name] = nc.dram_tensor(name, shape, F32, kind="ExternalInput").ap()

    din("keys", [L, BL, KD])
    din("vals", [L, BL, VD])
    din("rpe", [L, BL])
    din("step_rep", [B, 1])
    din("state", [BL, SDIM])
    din("lat", [BL, LAT])
    din("sel", [BL, B])
    din("W_state", [SDIM, MEMB])
    din("b_state", [128, 2])
    din("Wcq1", [KD, HID])
    din("bcq1", [128, 4])
    din("Wcq2", [HID, KD])
    din("bcq2", [128, 4])
    din("Wq", [KD, H * KD])
    din("bq", [128, 32])
    din("Wagg", [H * VD, VD])
    din("bagg", [128, 4])
    din("Wrk1", [VD, HID])
    din("brk1", [128, 4])
    din("Wrk2", [HID, RIMQ])
    din("brk2", [128, 4])
    din("Wrv1", [VD, HID])
    din("brv1", [128, 4])
    din("Wrv2", [HID, VD])
    din("brv2", [128, 4])
    io["out_key"] = nc.dram_tensor("out_key", [BL, RIMQ], F32,
                                   kind="ExternalOutput").ap()
    io["out_val"] = nc.dram_tensor("out_val", [BL, VD], F32,
                                   kind="ExternalOutput").ap()

    with tile.TileContext(nc) as tc, ExitStack() as ctx:
        _emit(nc, tc, ctx, io)
    nc.compile()
    return nc


def _rsb(bias, nch):
    return np.ascontiguousarray(
        np.asarray(bias, np.float32).reshape(nch, 128).T)


def _shard(inputs):
    f = lambda x: np.asarray(x, np.float32)
    keys, vals, rpe = f(inputs["keys"]), f(inputs["vals"]), f(inputs["rpe_mod"])
    step = np.asarray(inputs["step"]).astype(np.float32)
    state, lat = f(inputs["state"]), f(inputs["task_inference_latent"])
    sel = np.ascontiguousarray(
        np.repeat(np.eye(BL, dtype=np.float32), BL, axis=1) * RSQK)
    shared = {
        "sel": sel,
        "W_state": f(inputs["W_state"]), "b_state": _rsb(inputs["b_state"], 2),
        "Wcq1": f(inputs["Wcq1"]), "bcq1": _rsb(inputs["bcq1"], 4),
        "Wcq2": f(inputs["Wcq2"]), "bcq2": _rsb(inputs["bcq2"], 4),
        "Wq": f(inputs["Wq"]), "bq": _rsb(inputs["bq"], 32),
        "Wagg": f(inputs["Wagg"]), "bagg": _rsb(inputs["bagg"], 4),
        "Wrk1": f(inputs["Wrk1"]), "brk1": _rsb(inputs["brk1"], 4),
        "Wrk2": f(inputs["Wrk2"]), "brk2": _rsb(inputs["brk2"], 4),
        "Wrv1": f(inputs["Wrv1"]), "brv1": _rsb(inputs["brv1"], 4),
        "Wrv2": f(inputs["Wrv2"]), "brv2": _rsb(inputs["brv2"], 4),
    }
    in_maps = []
    for m in range(NCORES):
        b0 = m * BL
        in_maps.append({
            "keys": np.ascontiguousarray(keys[:, b0:b0 + BL, :]),
            "vals": np.ascontiguousarray(vals[:, b0:b0 + BL, :]),
            "rpe": np.ascontiguousarray(rpe[:, b0:b0 + BL, 0]),
            "step_rep": np.ascontiguousarray(
                np.repeat(step[b0:b0 + BL], H)[:, None]),
            "state": np.ascontiguousarray(state[b0:b0 + BL]),
            "lat": np.ascontiguousarray(lat[b0:b0 + BL]),
            **shared,
        })
    return in_maps


def kernel(**inputs):
    nc = _CACHE.get("nc")
    if nc is None:
        nc = _CACHE["nc"] = _build()
    in_maps = _shard(inputs)
    res = run_bass_kernel_spmd(nc, in_maps, list(range(NCORES)),
                               **_CACHE.get("run_kwargs", {}))
    _CACHE["last_result"] = res
    ok = np.concatenate([res.results[m]["out_key"] for m in range(NCORES)], 0)
    ov = np.concatenate([res.results[m]["out_val"] for m in range(NCORES)], 0)
    return ok[:, None, :], ov[:, None, :]
